# revision 1
# baseline (speedup 1.0000x reference)
"""Trainium2 Bass kernel: two-layer LIF spiking network scan.

Model (per timestep t, batch row b):
    h1 = x_t @ W1.T + b1            # [B, 32]
    v1 = v1 + (h1 - v1)/2           # tau = 2
    s1 = (v1 >= 1);  v1 *= (1-s1)   # hard reset
    h2 = s1 @ W2.T + b2             # [B, 1]
    v2 = v2 + (h2 - v2)/2
    s2 = (v2 >= 1);  v2 *= (1-s2)
    out = sum of s2 over t in [T - T//4, T)

Kernel strategy (pure data parallel over batch, 8 cores x 512 rows):
  - batch rows on the 128 SBUF partitions, 4 groups of 128 rows in the
    free dimension; the whole x shard (16 MiB) lives in SBUF.
  - sequential loop over T with fused custom DVE ops; the layer-1 state
    kept is the PRE-reset potential u (so spikes are just u >= 1):
      FMA2   c = x0*(W1[:,0]/2) + x1*(W1[:,1]/2)      (one op per group)
      LIF1   u' = (u < 1) ? 0.5*u + c : c             (decay + hard reset)
      SDS    prefix-sum along free of (u' >= 1)*W2h -> per-group spike
             dot via a strided difference of the prefix sums
  - the strided difference and the tiny layer-2 LIF chain run on the
    gpsimd engine, decoupled through an 8-slot scan ring.
"""

import numpy as np

B, T, I, H, O = 4096, 4096, 2, 32, 1
N_CORES = 8
B_CORE = B // N_CORES          # 512
G = B_CORE // 128              # 4 groups

_cache = {}


# ----------------------------------------------------------------- custom ops
def _register_custom_ops():
    """Register our custom DVE ops in the process-global registry (idempotent)."""
    import concourse.dve_ops as dve_ops_mod
    from concourse.dve_ops import DveOp
    from concourse.dve_spec import (
        Spec, Src0, Src1, C0, C1, C2, Zero, One,
        select, eq, lower, AluOp, scan, _has_src1,
    )
    from concourse.dve_uop import DveOpSpec

    if "ANT_SNN_FMA2" in dve_ops_mod._SUB_OPCODE_FOR_NAME:
        return

    def _ref_fma2(in0, in1, s0, s1, imm2):
        return (in0 * s0 + in1 * s1).astype(np.float32)

    def _ref_lif1(in0, in1, s0, s1, imm2):
        # state is the pre-reset potential u: u' = (u<1) ? 0.5u + c : c
        return np.where(
            in0 < 1.0, (in0 * np.float32(0.5)) + in1, in1
        ).astype(np.float32)

    def _ref_sds(in0, in1, s0, s1, imm2):
        # prefix sums of (u >= 1) * w2h along the free dim
        contrib = np.where(in0 < 1.0, np.float32(0.0), in1)
        return np.cumsum(contrib.astype(np.float32), axis=-1, dtype=np.float32)

    specs = [
        ("ANT_SNN_FMA2", Spec(body=Src0 * C0 + Src1 * C1, reference=_ref_fma2)),
        (
            "ANT_SNN_LIF1",
            Spec(
                body=select(Src0 < One, Src0 * C0 + Src1, Src1),
                reference=_ref_lif1,
            ),
        ),
        (
            "ANT_SNN_SDS",
            Spec(
                body=scan(AluOp.ADD, select(Src0 < One, Zero, Src1)),
                reference=_ref_sds,
            ),
        ),
    ]

    ops = {}
    for name, spec in specs:
        row = 1 + len(dve_ops_mod.OPS)
        sha = {}
        for ver in ("v3", "v4"):
            try:
                s = DveOpSpec(
                    name=name,
                    opcode=row,
                    uops=lower(spec, ver=ver),
                    rd1_en=_has_src1(spec),
                )
                sha[ver] = s.sha(ver)
            except Exception:
                pass
        op = DveOp(name, spec, subdim=False, uops_sha=sha)
        dve_ops_mod.OPS.append(op)
        dve_ops_mod.CUSTOM_DVE_SPECS[name] = spec
        dve_ops_mod._SUB_OPCODE_FOR_NAME[name] = row
        ops[name] = op
    return ops


def _get_ops():
    import concourse.dve_ops as dve_ops_mod

    _register_custom_ops()
    by_name = {op.name: op for op in dve_ops_mod.OPS}
    return (
        by_name["ANT_SNN_FMA2"],
        by_name["ANT_SNN_LIF1"],
        by_name["ANT_SNN_SDS"],
    )


# ----------------------------------------------------------------- bass build
def build_nc(t_steps=T, decision_start=None, has_b1=False, has_b2=False):
    """Build the per-core Bass program (SPMD; all cores run the same NEFF)."""
    import concourse.bass as bass
    import concourse.mybir as mybir

    OP_FMA2, OP_LIF1, OP_SDS = _get_ops()
    A = mybir.AluOpType
    f32 = mybir.dt.float32

    if decision_start is None:
        decision_start = max(t_steps - t_steps // 4, t_steps // 2)

    # Same-engine RAW hazards are safe on HW (per-op DVE pipeline drain);
    # the CoreSim race detector would flag them, so turn it off.
    nc = bass.Bass(detect_race_conditions=False)

    xs = nc.declare_dram_parameter("xs", [B_CORE, t_steps * I], f32, isOutput=False)
    wc0b = nc.declare_dram_parameter("wc0b", [128, H], f32, isOutput=False)
    wc1b = nc.declare_dram_parameter("wc1b", [128, H], f32, isOutput=False)
    w2hb = nc.declare_dram_parameter("w2hb", [128, G * H], f32, isOutput=False)
    k2b = nc.declare_dram_parameter("k2b", [128, 1], f32, isOutput=False)
    b1hb = nc.declare_dram_parameter("b1hb", [128, G * H], f32, isOutput=False)
    out = nc.declare_dram_parameter("out", [128, G], f32, isOutput=True)

    xlen = t_steps * I
    FW = G * H  # 128 free width for the fused tiles

    x_sbuf = nc.alloc_sbuf_tensor("x_sbuf", [128, G * xlen], f32).ap()
    wc0 = nc.alloc_sbuf_tensor("wc0", [128, H], f32).ap()
    wc1 = nc.alloc_sbuf_tensor("wc1", [128, H], f32).ap()
    w2h = nc.alloc_sbuf_tensor("w2h", [128, FW], f32).ap()
    b1h = nc.alloc_sbuf_tensor("b1h", [128, FW], f32).ap()
    k2 = nc.alloc_sbuf_tensor("k2", [128, 1], f32).ap()
    NS = 8  # scan ring depth (DVE->gpsimd decoupling, in steps)
    SW = FW + 4  # scan slot width
    S0 = nc.alloc_sbuf_tensor("S0", [128, FW], f32).ap()
    S1 = nc.alloc_sbuf_tensor("S1", [128, FW], f32).ap()
    cbuf = nc.alloc_sbuf_tensor("cbuf", [128, FW], f32).ap()
    scanring = nc.alloc_sbuf_tensor("scanring", [128, NS * SW], f32).ap()
    red4 = nc.alloc_sbuf_tensor("red4", [128, G], f32).ap()
    y2 = nc.alloc_sbuf_tensor("y2", [128, G], f32).ap()
    u2 = nc.alloc_sbuf_tensor("u2", [128, G], f32).ap()
    q2 = nc.alloc_sbuf_tensor("q2", [128, G], f32).ap()
    s2t = nc.alloc_sbuf_tensor("s2t", [128, G], f32).ap()
    accA = nc.alloc_sbuf_tensor("accA", [128, G], f32).ap()
    accB = nc.alloc_sbuf_tensor("accB", [128, G], f32).ap()
    acc_pp = [accA, accB]
    S_pp = [S0, S1]

    # x is streamed in NX time-chunks so the step loop starts after the
    # first chunk instead of the full 16 MiB load. Per-chunk semaphores:
    # a single completion-count semaphore could be satisfied out of order
    # across the 16 DMA queues.
    NX = 16 if t_steps % 16 == 0 else 1
    xchunk = t_steps // NX

    with (
        nc.semaphore("dma_sem") as dma_sem,
        nc.semaphore("d2g") as d2g,
        nc.semaphore("g2d") as g2d,
        nc.semaphore("g_done") as g_done,
        nc.Block() as block,
    ):
        sem_x = [nc.semaphore(f"sem_x{k}").__enter__() for k in range(NX)]

        @block.sync
        def _(sync):
            sync.dma_start(out=wc0[:], in_=wc0b[:]).then_inc(dma_sem, 16)
            sync.dma_start(out=wc1[:], in_=wc1b[:]).then_inc(dma_sem, 16)
            sync.dma_start(out=w2h[:], in_=w2hb[:]).then_inc(dma_sem, 16)
            sync.dma_start(out=k2[:], in_=k2b[:]).then_inc(dma_sem, 16)
            sync.dma_start(out=b1h[:], in_=b1hb[:]).then_inc(dma_sem, 16)
            for k in range(NX):
                for g in range(G):
                    sync.dma_start(
                        out=x_sbuf[
                            :,
                            g * xlen + k * xchunk * I : g * xlen
                            + (k + 1) * xchunk * I,
                        ],
                        in_=xs[
                            g * 128 : (g + 1) * 128,
                            k * xchunk * I : (k + 1) * xchunk * I,
                        ],
                    ).then_inc(sem_x[k], 16)
            sync.wait_ge(g_done, 1)
            sync.dma_start(out=out[:, :], in_=acc_pp[(t_steps - 1) % 2][:]).then_inc(
                dma_sem, 16
            )
            sync.wait_ge(dma_sem, 16 * 6)

        def scan_slot(t):
            base = (t % NS) * SW
            return (
                scanring[:, base + 1 : base + FW + 1],  # scan output
                scanring[:, base + H : base + FW + 1 : H],  # hi taps
                scanring[:, base : base + FW : H],  # lo taps
            )

        @block.vector
        def _(vector):
            vector.memset(S_pp[0][:], 0.0)
            vector.memset(scanring[:], 0.0)
            vector.memset(y2[:], 0.0)
            vector.memset(acc_pp[0][:], 0.0)
            vector.memset(acc_pp[1][:], 0.0)
            vector.wait_ge(dma_sem, 16 * 5)  # weight tiles
            for t in range(t_steps):
                src = S_pp[t % 2]
                dst = S_pp[1 - t % 2]
                if t % xchunk == 0:
                    vector.wait_ge(sem_x[t // xchunk], 16 * G)
                if t % 4 == 0 and t >= 8:
                    vector.wait_ge(g2d, t // 4 - 1)
                for g in range(G):
                    col = g * xlen + I * t
                    vector._custom_dve(
                        OP_FMA2,
                        out=cbuf[:, g * H : (g + 1) * H],
                        in0=wc0[:],
                        in1=wc1[:],
                        s0=x_sbuf[:, col : col + 1],
                        s1=x_sbuf[:, col + 1 : col + 2],
                    )
                if has_b1:
                    vector.tensor_tensor(
                        out=cbuf[:], in0=cbuf[:], in1=b1h[:], op=A.add
                    )
                vector._custom_dve(
                    OP_LIF1, out=dst[:], in0=src[:], in1=cbuf[:], s0=0.5
                )
                sout, _, _ = scan_slot(t)
                vector._custom_dve(
                    OP_SDS, out=sout, in0=dst[:], in1=w2h[:]
                ).then_inc(d2g, 1)

        @block.gpsimd
        def _(gpsimd):
            # Pool-legal ops only: tensor_scalar (incl. dual/compare) and
            # tensor_tensor add/mult/subtract.
            for t in range(t_steps):
                gpsimd.wait_ge(d2g, t + 1)
                _, hi, lo = scan_slot(t)
                gpsimd.tensor_tensor(out=red4[:], in0=hi, in1=lo, op=A.subtract)
                gpsimd.tensor_tensor(out=u2[:], in0=red4[:], in1=y2[:], op=A.add)
                if has_b2:
                    gpsimd.tensor_scalar(u2[:], u2[:], k2[:], None, A.add)
                if t >= decision_start:
                    gpsimd.tensor_scalar(s2t[:], u2[:], 1.0, None, A.is_ge)
                    gpsimd.tensor_tensor(
                        out=acc_pp[t % 2][:],
                        in0=acc_pp[1 - t % 2][:],
                        in1=s2t[:],
                        op=A.add,
                    )
                # q2 = (u2 < 1) * 0.5  -> y2 = u2 * q2
                gpsimd.tensor_scalar(q2[:], u2[:], 1.0, 0.5, A.is_lt, A.mult)
                ins = gpsimd.tensor_tensor(out=y2[:], in0=u2[:], in1=q2[:], op=A.mult)
                if t % 4 == 3:
                    ins.then_inc(g2d, 1)
            gpsimd.tensor_scalar(q2[:], q2[:], 1.0, None, A.mult).then_inc(g_done, 1)

    # Populate .instr bytes for InstISA subclasses (custom DVE ops). Raw
    # Bass skips this pass; without it walrus fails with "ISA wrong length".
    mybir.codegen_inst_isa_subclasses(nc)
    return nc


def _host_tiles(W1, b1, W2, b2):
    wc0b = np.tile((W1[:, 0] * 0.5).astype(np.float32)[None, :], (128, 1))
    wc1b = np.tile((W1[:, 1] * 0.5).astype(np.float32)[None, :], (128, 1))
    w2hb = np.tile((W2[0, :] * 0.5).astype(np.float32)[None, :], (128, G))
    k2b = np.full((128, 1), 0.5 * float(b2[0]), np.float32)
    b1hb = np.tile((b1 * 0.5).astype(np.float32)[None, :], (128, G))
    return wc0b, wc1b, w2hb, k2b, b1hb


def kernel(x, W1, b1, W2, b2):
    from concourse.bass_utils import run_bass_kernel_spmd

    has_b1 = bool(np.any(np.asarray(b1) != 0))
    has_b2 = bool(np.any(np.asarray(b2) != 0))
    key = ("nc", T, has_b1, has_b2)
    if key not in _cache:
        _cache[key] = build_nc(T, has_b1=has_b1, has_b2=has_b2)
    nc = _cache[key]

    wc0b, wc1b, w2hb, k2b, b1hb = _host_tiles(
        np.asarray(W1), np.asarray(b1), np.asarray(W2), np.asarray(b2)
    )
    x = np.ascontiguousarray(np.asarray(x, np.float32))
    in_maps = []
    for c in range(N_CORES):
        shard = x[c * B_CORE : (c + 1) * B_CORE].reshape(B_CORE, T * I)
        in_maps.append(
            {
                "xs": shard,
                "wc0b": wc0b,
                "wc1b": wc1b,
                "w2hb": w2hb,
                "k2b": k2b,
                "b1hb": b1hb,
            }
        )

    res = run_bass_kernel_spmd(nc, in_maps, list(range(N_CORES)))
    # out[p, g] holds batch row g*128 + p of the core's shard
    outs = [
        np.asarray(res.results[c]["out"]).T.reshape(B_CORE) for c in range(N_CORES)
    ]
    return np.concatenate(outs).reshape(B, 1).astype(np.float32)

